# revision 5
# baseline (speedup 1.0000x reference)
"""Trainium2 Bass kernel for CapsNet dynamic routing (ClassCapsules).

Reference computation (B=256, R=1152, C=10, O=16, I=8, 3 routing iters):
    u_hat[b,r,c,o] = sum_i W[r,c,o,i] * x[b,r,i]
    b_ij = 0
    for it in 3:
        c_ij = softmax(b_ij, axis=1)                      # over c
        s = sum_r c_ij[r,c] * u_hat[b,r,c,o] + bias       # [B,C,O]
        v = squash(s)
        if it < 2:
            b_ij += mean_b sum_o u_hat[b,r,c,o] v[b,c,o]  # [R,C]
    return v[..., None]

u_hat ([B,R,C,O] = 189MB fp32) is never materialized.  Both routing
contractions are re-associated through the factorization
    s[b,co]    = x~[b,(ri)] @ (c∘W~)[(ri),(co)]
    agree[r,c] = sum_{i,o} W~[(ri),(co)] * G[(ri),(co)],
                 G = (1/B) x~^T v
with x~ = x viewed as [B, R*I] and W~ = W viewed as [R*I, C*O].

Distribution: COLLECTIVE-FREE full replication.  Measured on this part,
any collective chain costs ~40-70us of CC-engine setup/rendezvous before
the first transfer plus ~13us per AllReduce, all serialized with the
routing iterations; per-core HW exec time is measured per-core, so a
kernel with no cross-core sync pays neither setup nor launch skew.
Every core redundantly computes the full-batch routing state (s, v,
agree, c_ij) in fp16 matmuls (fp32 PSUM accumulation), and only the
final iteration narrows to the core's own 32-batch output shard.
"""

import os
import sys
import types

sys.path.insert(0, "/opt/trn_rl_repo")

# Shim antenv.axon_hooks (absent on this image) so BASS_TRACE=1 profiling
# works through run_bass_kernel_spmd's axon path.  Harmless when unused.
try:
    import antenv.axon_hooks  # noqa: F401
except ImportError:
    try:
        _hooks = types.ModuleType("antenv.axon_hooks")
        _hooks._hook = None
        _hooks.set_axon_ntff_profile_hook = lambda h: setattr(_hooks, "_hook", h)
        _hooks.get_axon_ntff_profile_hook = lambda: _hooks._hook
        sys.modules["antenv.axon_hooks"] = _hooks
        import antenv
        antenv.axon_hooks = _hooks
        from trn_agent_boot.trn_boot import _ntff_profile_via_ctypes
        _hooks.set_axon_ntff_profile_hook(
            _ntff_profile_via_ctypes("/opt/axon/libaxon_pjrt.so")
        )
    except Exception:
        pass

import numpy as np

import concourse.bacc as bacc
import concourse.bass as bass
import concourse.tile as tile
from concourse import mybir
import concourse.bass_utils as _bass_utils
from concourse.bass_utils import run_bass_kernel_spmd

if os.environ.get("BASS_TRACE"):
    _bass_utils.upload_artifacts = lambda tmpdir: ""  # no bucket access here

LAST_RESULT = None

F32 = mybir.dt.float32
F16 = mybir.dt.float16
ALU = mybir.AluOpType
ACT = mybir.ActivationFunctionType

B, R, C, O, I = 256, 1152, 10, 16, 8
CO = C * O                      # 160
N_CORES = 8
RI = R * I                      # 9216
NG = RI // 128                  # 72 groups of 128 (r,i) rows
GG = 8                          # dma/load granularity: 8 groups per chunk
NGG = NG // GG                  # 9 chunks
NB = B // 128                   # 2 batch partition chunks
B_SHARD = B // N_CORES          # 32 batches output per core
ITERS = 3
RPG = 128 // I                  # 16 r's per group
PB = 8                          # p9 reduce block (groups)

ITERS_RANGE = range(ITERS)


def _squash(nc, eps_sb, t, n_part, nb, pool, out_ap, name):
    """out = t * n2/((1+n2)*sqrt(n2+eps)); t: [n_part, nb, CO], reduce over o.

    out_ap must be an [n_part, nb, CO]-shaped AP.
    """
    nc_ = nb * C
    tf = t.rearrange("p nb co -> p (nb co)")
    sq = pool.tile([n_part, nb * CO], F32, tag="sq", name=f"sq_{name}")
    nc.vector.tensor_mul(sq, tf, tf)
    n2 = pool.tile([n_part, nc_], F32, tag="n2", name=f"n2_{name}")
    nc.vector.reduce_sum(
        n2, sq.rearrange("p (nb c o) -> p nb c o", nb=nb, c=C),
        axis=mybir.AxisListType.X,
    )
    rt = pool.tile([n_part, nc_], F32, tag="rt", name=f"rt_{name}")
    nc.scalar.activation(rt, n2, ACT.Sqrt, bias=eps_sb[:n_part])
    n2p1 = pool.tile([n_part, nc_], F32, tag="n2p1", name=f"n2p1_{name}")
    nc.vector.tensor_scalar_add(n2p1, n2, 1.0)
    den = pool.tile([n_part, nc_], F32, tag="den", name=f"den_{name}")
    nc.vector.tensor_mul(den, n2p1, rt)
    rec = pool.tile([n_part, nc_], F32, tag="rec", name=f"rec_{name}")
    nc.vector.reciprocal(rec, den)
    fac = pool.tile([n_part, nc_], F32, tag="fac", name=f"fac_{name}")
    nc.vector.tensor_mul(fac, n2, rec)
    fac_b = fac.rearrange(
        "p (nb c one) -> p nb c one", nb=nb, c=C
    ).broadcast_to([n_part, nb, C, O])
    nc.vector.tensor_tensor(
        out=out_ap.rearrange("p nb (c o) -> p nb c o", c=C),
        in0=t.rearrange("p nb (c o) -> p nb c o", c=C),
        in1=fac_b,
        op=ALU.mult,
    )


def build():
    nc = bacc.Bacc("TRN2", target_bir_lowering=False, debug=False,
                   num_devices=N_CORES)

    # fp16 inputs, host pre-packed so every SBUF partition reads one
    # contiguous DRAM block.
    xt_d = nc.dram_tensor("xt", [128, NG, B], F16, kind="ExternalInput")
    xb_d = nc.dram_tensor("xb", [NB, 128, NG, 128], F16, kind="ExternalInput")
    xo_d = nc.dram_tensor("xo", [128, NG, B_SHARD], F16, kind="ExternalInput")
    wg_d = nc.dram_tensor("wg", [128, NG, CO], F16, kind="ExternalInput")
    bias_d = nc.dram_tensor("biasf", [CO], F32, kind="ExternalInput")
    sel_d = nc.dram_tensor("sel", [128, RPG], F32, kind="ExternalInput")
    selT_d = nc.dram_tensor("selT", [RPG, 128], F32, kind="ExternalInput")
    y_d = nc.dram_tensor("y", [B_SHARD, CO], F32, kind="ExternalOutput")

    with tile.TileContext(nc) as tc:
        with (
            tc.tile_pool(name="singles", bufs=1) as singles,
            tc.tile_pool(name="cw_pool", bufs=2) as cw_pool,
            tc.tile_pool(name="work", bufs=2) as work,
            tc.tile_pool(name="small", bufs=2) as small,
            tc.tile_pool(name="psum_s", bufs=1, space="PSUM") as psum_s,
            tc.tile_pool(name="psum_g", bufs=2, space="PSUM") as psum_g,
            tc.tile_pool(name="psum_misc", bufs=1, space="PSUM") as psum_misc,
        ):
            # ---- small constants first ----
            biasb = singles.tile([128, CO], F32)
            nc.sync.dma_start(
                out=biasb,
                in_=bass.AP(tensor=bias_d, offset=0, ap=[[0, 128], [1, CO]]),
            )
            sel_sb = singles.tile([128, RPG], F32)
            nc.sync.dma_start(out=sel_sb, in_=sel_d[:, :])
            selT_sb = singles.tile([RPG, 128], F32)
            nc.sync.dma_start(out=selT_sb, in_=selT_d[:, :])
            eps_sb = singles.tile([128, 1], F32)
            nc.vector.memset(eps_sb, 1e-8)

            # ---- bulk loads, chunked for load/compute pipelining ----
            # W and XT feed iteration 0's s-matmuls; XB feeds G.
            WG = []                                    # 9 x [128, 8, CO]
            XT = []                                    # 9 x [128, 8, B]
            for gg in range(NGG):
                w_t = singles.tile([128, GG, CO], F16, tag=f"wg{gg}",
                                   name=f"wg_sb{gg}")
                nc.sync.dma_start(out=w_t, in_=wg_d[:, gg * GG:(gg + 1) * GG, :])
                WG.append(w_t)
                x_t = singles.tile([128, GG, B], F16, tag=f"xt{gg}",
                                   name=f"xt_sb{gg}")
                nc.sync.dma_start(out=x_t, in_=xt_d[:, gg * GG:(gg + 1) * GG, :])
                XT.append(x_t)
            XB = []                                    # [kb][gg] [128, 8, 128]
            for kb in range(NB):
                tiles = []
                for gg in range(NGG):
                    t = singles.tile([128, GG, 128], F16, tag=f"xb{kb}_{gg}",
                                     name=f"xb_sb{kb}_{gg}")
                    nc.sync.dma_start(
                        out=t, in_=xb_d[kb, :, gg * GG:(gg + 1) * GG, :]
                    )
                    tiles.append(t)
                XB.append(tiles)
            XO = singles.tile([128, NG, B_SHARD], F16)
            nc.sync.dma_start(out=XO, in_=xo_d[:, :, :])

            esr = None   # [16, 720 + 72]: exp(b_ij) ++ 1/sum_c exp(b_ij)

            for it in ITERS_RANGE:
                # ---- CW = c∘W~ (it>0); it=0 uses uniform c=0.1 folded
                # into the post-matmul scale.
                if it == 0:
                    CW = None
                else:
                    # Broadcast esr across partitions via PE: cp[p, col] =
                    # esr[p//8, col].  cols 0..719 = exp(b)[g,c],
                    # 720..791 = 1/sum_c exp(b) [g].
                    cp_sb = small.tile([128, NG * C + NG], F32, tag="cpart",
                                       name=f"cp_sb_{it}")
                    half = 400
                    for h, (lo, hi) in enumerate(((0, half),
                                                  (half, NG * C + NG))):
                        cp_ps = psum_misc.tile([128, half], F32, tag="cp",
                                               name=f"cp_ps_{it}_{h}")
                        nc.tensor.matmul(cp_ps[:, :hi - lo], selT_sb,
                                         esr[:, lo:hi], start=True, stop=True)
                        nc.scalar.copy(cp_sb[:, lo:hi], cp_ps[:, :hi - lo])
                    CW = cw_pool.tile([128, NG, CO], F16, tag="cw",
                                      name=f"cw_{it}")
                    for g in range(NG):
                        e_b = cp_sb[:, g * C:(g + 1) * C].rearrange(
                            "p (c one) -> p c one", one=1
                        ).broadcast_to([128, C, O])
                        nc.vector.scalar_tensor_tensor(
                            out=CW[:, g, :].rearrange("p (c o) -> p c o", c=C),
                            in0=WG[g // GG][:, g % GG, :].rearrange(
                                "p (c o) -> p c o", c=C),
                            scalar=cp_sb[:, NG * C + g:NG * C + g + 1],
                            in1=e_b,
                            op0=ALU.mult, op1=ALU.mult,
                        )

                if it < ITERS - 1:
                    # ---- s (full batch): [256,160] = x~^T @ CW ----
                    s_ps = [psum_s.tile([128, CO], F32, tag=f"s{kb}",
                                        name=f"s_ps{kb}_{it}")
                            for kb in range(NB)]
                    v_sb = work.tile([128, NB, CO], F16, tag="vsb",
                                     name=f"v_sb_{it}")
                    for kb in range(NB):
                        for g in range(NG):
                            cw_g = (WG[g // GG][:, g % GG, :] if it == 0
                                    else CW[:, g, :])
                            nc.tensor.matmul(
                                s_ps[kb],
                                XT[g // GG][:, g % GG,
                                            kb * 128:(kb + 1) * 128],
                                cw_g,
                                start=(g == 0),
                                stop=(g == NG - 1),
                            )
                        t = work.tile([128, 1, CO], F32, tag="t",
                                      name=f"t_{it}_{kb}")
                        bias_b = biasb.rearrange(
                            "p (one co) -> p one co", one=1
                        )
                        nc.vector.scalar_tensor_tensor(
                            out=t,
                            in0=s_ps[kb].rearrange("p (one co) -> p one co",
                                                   one=1),
                            scalar=(0.1 if it == 0 else 1.0),
                            in1=bias_b, op0=ALU.mult, op1=ALU.add,
                        )
                        _squash(nc, eps_sb, t, 128, 1, work,
                                v_sb[:, kb:kb + 1, :], f"{it}_{kb}")

                    # ---- G = (1/B) x~^T v ; agree = sum_io W∘G ----
                    Q_all = small.tile([128, NG * C], F32, tag="qall",
                                       name=f"qall_{it}")
                    for g in range(NG):
                        g_ps = psum_g.tile([128, CO], F32, tag="gps",
                                           name=f"g_ps_{it}_{g}")
                        for kb in range(NB):
                            nc.tensor.matmul(
                                g_ps,
                                XB[kb][g // GG][:, g % GG, :],
                                v_sb[:, kb, :],
                                start=(kb == 0),
                                stop=(kb == NB - 1),
                            )
                        if g % PB == 0:
                            p9 = work.tile([128, PB, CO], F32, tag="p9",
                                           name=f"p9_{it}_{g // PB}")
                        nc.vector.scalar_tensor_tensor(
                            out=p9[:, g % PB, :], in0=g_ps, scalar=1.0 / B,
                            in1=WG[g // GG][:, g % GG, :],
                            op0=ALU.mult, op1=ALU.mult,
                        )
                        if g % PB == PB - 1:
                            lo = g - (PB - 1)
                            nc.vector.reduce_sum(
                                Q_all[:, lo * C:(g + 1) * C],
                                p9.rearrange("p g (c o) -> p (g c) o", c=C),
                                axis=mybir.AxisListType.X,
                            )

                    # ---- agree (i-sum via sel matmul), esr update ----
                    esr_prev = esr
                    esr = small.tile([RPG, NG * C + NG], F32, tag="esr",
                                     name=f"esr_{it}")
                    half_a = 512
                    for h, (lo, hi) in enumerate(((0, half_a),
                                                  (half_a, NG * C))):
                        agree_ps = psum_misc.tile([RPG, half_a], F32,
                                                  tag="agree",
                                                  name=f"agree_{it}_{h}")
                        nc.tensor.matmul(agree_ps[:, :hi - lo], sel_sb,
                                         Q_all[:, lo:hi],
                                         start=True, stop=True)
                        if it == 0:
                            nc.scalar.activation(esr[:, lo:hi],
                                                 agree_ps[:, :hi - lo],
                                                 ACT.Exp)
                        else:
                            eexp = small.tile([RPG, half_a], F32, tag="eexp",
                                              name=f"eexp_{it}_{h}")
                            nc.scalar.activation(eexp[:, :hi - lo],
                                                 agree_ps[:, :hi - lo],
                                                 ACT.Exp)
                            nc.vector.tensor_mul(
                                esr[:, lo:hi], esr_prev[:, lo:hi],
                                eexp[:, :hi - lo]
                            )
                    den = small.tile([RPG, NG], F32, tag="sden",
                                     name=f"den_{it}")
                    nc.vector.reduce_sum(
                        den,
                        esr[:, :NG * C].rearrange("p (g c) -> p g c", g=NG),
                        axis=mybir.AxisListType.X,
                    )
                    nc.vector.reciprocal(esr[:, NG * C:], den)
                else:
                    # ---- final iter: s for own 32-batch shard only ----
                    s2_ps = psum_s.tile([128, CO], F32, tag="s0",
                                        name="s2_ps")
                    for g in range(NG):
                        nc.tensor.matmul(
                            s2_ps[:B_SHARD, :],
                            XO[:, g, :],
                            CW[:, g, :],
                            start=(g == 0),
                            stop=(g == NG - 1),
                        )
                    t2 = work.tile([B_SHARD, 1, CO], F32, tag="ft")
                    bias_b1 = biasb[:B_SHARD, :].rearrange(
                        "p (one co) -> p one co", one=1
                    )
                    nc.vector.scalar_tensor_tensor(
                        out=t2,
                        in0=s2_ps[:B_SHARD, :].rearrange(
                            "p (one co) -> p one co", one=1),
                        scalar=1.0,
                        in1=bias_b1, op0=ALU.mult, op1=ALU.add,
                    )
                    v2 = work.tile([B_SHARD, 1, CO], F32, tag="v2")
                    _squash(nc, eps_sb, t2, B_SHARD, 1, work, v2[:, :, :],
                            "fin")
                    nc.sync.dma_start(
                        out=y_d[:, :],
                        in_=v2.rearrange("p one co -> p (one co)")
                    )

    nc.compile()
    return nc


_NC = None


def kernel(x: np.ndarray, W: np.ndarray, bias: np.ndarray) -> np.ndarray:
    global _NC
    if _NC is None:
        _NC = build()

    x = np.ascontiguousarray(x, dtype=np.float32)
    W = np.ascontiguousarray(W, dtype=np.float32)
    bias = np.ascontiguousarray(bias, dtype=np.float32)

    xf = x.reshape(B, RI)
    # XT: [p, g, b] with ri = g*128 + p
    xt9 = np.ascontiguousarray(
        xf.T.reshape(NG, 128, B).transpose(1, 0, 2).astype(np.float16)
    )
    # XB: [kb, p_b, g, col] with b = kb*128 + p_b, ri = g*128 + col
    xb9 = np.ascontiguousarray(
        xf.reshape(NB, 128, NG, 128).astype(np.float16)
    )
    # W~: [(r i), (c o)] -> [p, g, co]
    wk = W.transpose(0, 3, 1, 2).reshape(RI, CO)
    wg9 = np.ascontiguousarray(
        wk.reshape(NG, 128, CO).transpose(1, 0, 2).astype(np.float16)
    )
    biasf = bias.reshape(CO)
    sel = np.zeros((128, RPG), dtype=np.float32)
    sel[np.arange(128), np.arange(128) // I] = 1.0
    selT = np.ascontiguousarray(sel.T)

    in_maps = []
    for k in range(N_CORES):
        xo = np.ascontiguousarray(
            xt9[:, :, k * B_SHARD:(k + 1) * B_SHARD]
        )
        in_maps.append({
            "xt": xt9,
            "xb": xb9,
            "xo": xo,
            "wg": wg9,
            "biasf": biasf,
            "sel": sel,
            "selT": selT,
        })

    global LAST_RESULT
    res = run_bass_kernel_spmd(
        _NC, in_maps, list(range(N_CORES)),
        trace=bool(os.environ.get("BASS_TRACE")),
    )
    LAST_RESULT = res
    v = np.concatenate([res.results[k]["y"] for k in range(N_CORES)], axis=0)
    return v.reshape(B, C, O)[..., None].astype(np.float32)
